# revision 7
# baseline (speedup 1.0000x reference)
"""Trainium2 Bass kernel for additive-attention pooling.

reference math:
    scores[b,t] = tanh(q[b]) @ vw_a + tanh(c[b,t]) @ vw_b
    attn        = softmax(where(mask<1, -1e10, scores), axis=t)
    out[b,e]    = sum_t attn[b,t] * c[b,t,e]

Softmax is shift-invariant and the query term is constant over t, so the
output does not depend on `query` or `v_w[:E]`.  Per batch row, one pass
over context:
    th   = tanh(c)                      (ACT, bf16 out)
    s_u  = sum_e (th + mb_u)*w2         (DVE scalar_tensor_tensor with
                                         accum_out; mb_u = (m-1)*1e9/sum(w2)
                                         rides the per-partition scalar slot,
                                         pushing masked rows to score -1e9)
    p_u  = exp(s_u)                     (ACT, f32r out; masked -> 0.0)
    num  = sum_t p_t*c_t                (PE float32r matmuls into PSUM:
                                         1 cycle/col at >=256 cols, vs 4 for
                                         f32 and ~1.6 for v1's strided-bf16)
    den  = sum_t p_t                    (ones.T @ pall matmul + free-dim
                                         reduce)
    out  = num / den                    (host side, 16x768 divides)

Measured engine facts driving this design (HW probes, this container):
  - tensor_tensor_reduce (native TTR) WEDGES the device -> unusable.
  - scalar_tensor_tensor + accum: 958ns + 81ns accum-read per [128,768]
    slice, dtype-independent (1x uop only) -> DVE dot = ~66us/core, the
    #2 engine after DMA (~70us HBM floor for 25.2MB/core f32).
  - affine_mul_reduce works but costs the same and TT+reduce is slower
    (reduce has no fast uop: 952ns at any dtype).
  - f32r end-to-end (DMA-tagged tiles, ACT exp->f32r, fused-ldw matmul)
    is correct on HW (1.4e-4) and fast on PE.
  - v1's per-tile chain tanh->dot->exp->matmul serialized via the ACT
    FIFO (exp head-of-line blocks the next tanh).  Here exp+matmuls are
    issued TWO tiles behind (skew-2), so exp's DVE dependency is always
    ready before it reaches the ACT queue head.

t-tiles pack J=4 context rows per partition ([128 x (j e)], 12KB
contiguous per partition per DMA).  First/last tiles are smaller to
shorten pipeline fill and drain.

Sharding: pure data parallel, batch 16 -> 2 per core on 8 cores; w2
replicated.  No collectives needed.
"""

import sys

for _p in ("/opt/trn_rl_repo", "/root/.axon_site/_ro/trn_rl_repo"):
    if _p not in sys.path:
        sys.path.append(_p)

import numpy as np

B, T, E = 16, 4096, 768
NCORES = 8
BPC = B // NCORES  # batches per core
P = 128            # partitions per tile
NEG_BIG = 1.0e9    # exp(-1e9) == 0.0

# per-batch tile schedule: (t0, nrows).  nrows/128 = rows per partition.
# batch 0 ramps up (short first tiles so compute starts early), the last
# batch ramps down (short last tiles so the post-DMA tail is short).
_RAMP = [(0, 128), (128, 128), (256, 256)]
_BODY = [(512 * k, 512) for k in range(1, 8)]
SCHED = [
    _RAMP + _BODY,
    [(t, 512) for t in range(0, 3584, 512)]
    + [(3584, 256), (3840, 128), (3968, 128)],
]
UNITS_PER_BATCH = T // P  # 32 mbias columns (128-row units) per batch

_cache = {}


def _build_program():
    from collections import deque

    import concourse.tile as tile
    from concourse import bacc, mybir

    f32 = mybir.dt.float32
    f32r = mybir.dt.float32r
    bf16 = mybir.dt.bfloat16
    AF = mybir.ActivationFunctionType
    ALU = mybir.AluOpType

    nc = bacc.Bacc(
        "TRN2",
        target_bir_lowering=False,
        debug=False,
        enable_asserts=False,
        num_devices=NCORES,
    )
    ctx_d = nc.dram_tensor("ctx", [BPC, T, E], f32r, kind="ExternalInput")
    w2_d = nc.dram_tensor("w2b", [P, E], bf16, kind="ExternalInput")
    mb_d = nc.dram_tensor(
        "mbias", [P, BPC * UNITS_PER_BATCH], f32, kind="ExternalInput"
    )
    ones_d = nc.dram_tensor("onesr", [P, 1], f32r, kind="ExternalInput")
    out_d = nc.dram_tensor("out", [BPC, E + 1], f32, kind="ExternalOutput")

    with tile.TileContext(nc) as tc:
        with (
            tc.tile_pool(name="const", bufs=1) as const_pool,
            tc.tile_pool(name="cin", bufs=8) as c_pool,
            tc.tile_pool(name="tanh", bufs=5) as t_pool,
            tc.tile_pool(name="small", bufs=8) as s_pool,
            tc.tile_pool(name="pall", bufs=2) as pall_pool,
            tc.tile_pool(name="outp", bufs=2) as o_pool,
            tc.tile_pool(name="pacc", bufs=2, space="PSUM") as pa_pool,
            tc.tile_pool(name="pden", bufs=2, space="PSUM") as pd_pool,
        ):
            def load_tile(b, t0, jt):
                c = c_pool.tile([P, 4 * E], f32r)
                nc.sync.dma_start(
                    c[:, 0:jt * E].rearrange("p (j e) -> p j e", j=jt),
                    ctx_d[b, t0:t0 + P * jt, :].rearrange(
                        "(p j) e -> p j e", j=jt
                    ),
                )
                return c

            # first context tile ahead of the (small) constant DMAs so the
            # SDMA engines start streaming immediately
            pre0 = load_tile(0, SCHED[0][0][0], SCHED[0][0][1] // P)

            w2b = const_pool.tile([P, E], bf16)
            nc.sync.dma_start(w2b[:], w2_d[:])
            mbias = const_pool.tile([P, BPC * UNITS_PER_BATCH], f32)
            nc.sync.dma_start(mbias[:], mb_d[:])
            ones = const_pool.tile([P, 1], f32r)
            nc.sync.dma_start(ones[:], ones_d[:])

            pre1 = load_tile(0, SCHED[0][1][0], SCHED[0][1][1] // P)
            preloaded = {0: pre0, 1: pre1}

            unit = 0  # global mbias column counter, schedule order
            for b in range(BPC):
                acc = pa_pool.tile([1, E], f32)       # sum_t p_t * c_t
                den = pd_pool.tile([1, UNITS_PER_BATCH], f32)
                pall = pall_pool.tile([P, UNITS_PER_BATCH], f32r)
                ntiles = len(SCHED[b])
                bcol = 0

                def flush(c, s2, jt, col, first, last):
                    nc.scalar.activation(
                        pall[:, col:col + jt], s2[:, 0:jt], AF.Exp
                    )
                    for jj in range(jt):
                        lhsT = pall[:, col + jj:col + jj + 1]
                        st = first and jj == 0
                        sp = last and jj == jt - 1
                        nc.tensor.matmul(
                            acc[:, 0:512], lhsT=lhsT,
                            rhs=c[:, jj * E:jj * E + 512],
                            start=st, stop=sp,
                        )
                        nc.tensor.matmul(
                            acc[:, 512:E], lhsT=lhsT,
                            rhs=c[:, jj * E + 512:(jj + 1) * E],
                            start=st, stop=sp,
                        )

                pend = deque()
                for ti, (t0, nr) in enumerate(SCHED[b]):
                    jt = nr // P
                    c = preloaded.pop(ti, None) if b == 0 else None
                    if c is None:
                        c = load_tile(b, t0, jt)
                    th = t_pool.tile([P, 4 * E], bf16)
                    nc.scalar.activation(
                        th[:, 0:jt * E].rearrange("p (j e) -> p j e", j=jt),
                        c[:, 0:jt * E].bitcast(f32).rearrange(
                            "p (j e) -> p j e", j=jt
                        ),
                        AF.Tanh,
                    )
                    s2 = s_pool.tile([P, 4], f32)
                    for jj in range(jt):
                        sl = slice(jj * E, (jj + 1) * E)
                        nc.vector.scalar_tensor_tensor(
                            th[:, sl],
                            th[:, sl],
                            mbias[:, unit + jj:unit + jj + 1],
                            w2b[:],
                            ALU.add,
                            ALU.mult,
                            accum_out=s2[:, jj:jj + 1],
                        )
                    pend.append((c, s2, jt, bcol, ti == 0, ti == ntiles - 1))
                    if len(pend) > 2:
                        flush(*pend.popleft())
                    unit += jt
                    bcol += jt

                while pend:
                    flush(*pend.popleft())

                # denominator: ones.T @ pall -> [1, 32] psum, then reduce
                nc.tensor.matmul(
                    den[:], lhsT=ones[:],
                    rhs=pall[:], start=True, stop=True,
                )
                out_sb = o_pool.tile([1, E + 1], f32)
                nc.scalar.copy(out_sb[:, 0:E], acc[:])
                nc.vector.tensor_reduce(
                    out_sb[:, E:E + 1], den[:],
                    mybir.AxisListType.X, ALU.add,
                )
                nc.sync.dma_start(out_d[b:b + 1, :], out_sb[:])

    nc.compile()
    return nc


def _get_program():
    if "nc" not in _cache:
        _cache["nc"] = _build_program()
    return _cache["nc"]


def build_in_maps(context, mask, v_w):
    import ml_dtypes

    w2 = np.asarray(v_w[E:], dtype=np.float32)
    w2bf = w2.astype(ml_dtypes.bfloat16)
    w2b = np.ascontiguousarray(np.broadcast_to(w2bf, (P, E)))
    # mask bias rides the STT scalar slot, added to every element BEFORE
    # the multiply by w2: sum((th+mb)*w2) = score + mb*sum(w2), so scale
    # mb so masked rows land at exactly -1e9.
    r = np.float32(NEG_BIG) / w2bf.astype(np.float32).sum(dtype=np.float32)
    mb_full = (np.asarray(mask, dtype=np.float32) - 1.0) * r
    in_maps = []
    for i in range(NCORES):
        cols = []
        for b in range(BPC):
            row = mb_full[i * BPC + b]
            for (t0, nr) in SCHED[b]:
                jt = nr // P
                for jj in range(jt):
                    cols.append(row[t0 + jj:t0 + nr:jt])
        mbias = np.ascontiguousarray(np.stack(cols, axis=1))  # [P, 64]
        in_maps.append(
            {
                "ctx": np.ascontiguousarray(context[i * BPC:(i + 1) * BPC]),
                "w2b": w2b,
                "mbias": mbias,
                "onesr": np.ones((P, 1), dtype=np.float32),
            }
        )
    return in_maps


def kernel(query, context, mask, v_w):
    import time
    from concourse.bass_utils import run_bass_kernel_spmd

    nc = _get_program()
    in_maps = build_in_maps(context, mask, v_w)
    last_err = None
    for attempt in range(3):
        try:
            res = run_bass_kernel_spmd(nc, in_maps, list(range(NCORES)))
            raw = np.concatenate(
                [res.results[i]["out"] for i in range(NCORES)], axis=0
            )
            return raw[:, :E] / raw[:, E:E + 1]
        except Exception as e:  # transient axon/device hiccups
            last_err = e
            time.sleep(5)
    raise last_err
